# revision 3
# baseline (speedup 1.0000x reference)
"""Trainium2 Bass kernel for nn_AttentionMemory (sparse_attention).

reference:
    mkf = mk.reshape(B, CK, HW); qkf = qk.reshape(B, CK, HW)
    affinity[b, m, q] = (-|mk_m|^2 + 2 mk_m.qk_q - |qk_q|^2) / sqrt(CK)
    out = softmax(affinity, axis=m)

Math: softmax over m drops any term constant in m, so -|qk_q|^2 vanishes:
    out[b, :, q] = softmax_m( (2 mk_m.qk_q - |mk_m|^2) / 8 )
The per-m bias -|mk_m|^2 is folded into the matmul as an augmented K row:
    lhsT = [mk ; -|mk_m|^2/2] (65 x m), rhs = [qk ; 1] (65 x q)
    z[m, q] = mk_m.qk_q - a_m/2        exp arg = z/4 (ACT scale operand)
so every Activation is bias-free and m-tiles can share one instruction.

Distribution: data parallel over B=16 -> 2 batches/core on 8 cores.

Per-core pipeline, per (batch, 512-wide q chunk):
  - 18 m-tile matmuls (fp32r, K=65) -> PSUM pairs [128, 2*512]
  - one Exp per PAIR of tiles (scale=0.25) -> e_sb bf16 [128, 18, 512]
  - column sums via all-ones bf16 lhsT matmuls accumulated over tiles
  - reciprocal -> r; bf16 16-bit multiplies e*r -> f16 out tiles
  - one store DMA per SIX m-tiles (contiguous 6*128 rows of out)
"""

import numpy as np
from contextlib import ExitStack

import concourse.bass as bass
import concourse.tile as tile
from concourse import bacc, mybir

B, CK, H, W = 16, 64, 36, 64
HW = H * W                 # 2304
NCORES = 8
BL = B // NCORES           # 2 batches per core
MT = HW // 128             # 18 m-tiles
KA = CK + 1                # augmented K (bias row)
QCH = 512                  # q chunk = one PSUM bank
F32 = mybir.dt.float32
F32R = mybir.dt.float32r
BF16 = mybir.dt.bfloat16
F16 = mybir.dt.float16
AF = mybir.ActivationFunctionType
ALU = mybir.AluOpType

CHUNKS = [(0, 512), (512, 512), (1024, 512), (1536, 512), (2048, 256)]
CHUNKS_B0 = [(0, 256), (256, 512), (768, 512), (1280, 512), (1792, 512)]
# b1 tapers: the final chunks shrink so the end-of-kernel store drain is short
CHUNKS_B1 = [(0, 512), (512, 512), (1024, 512), (1536, 384), (1920, 256), (2176, 128)]


def _build_kernel(tc: tile.TileContext, out_ext, mk_ext, qk_ext):
    nc = tc.nc
    with ExitStack() as ctx:
        singles = ctx.enter_context(tc.tile_pool(name="singles", bufs=1))
        mk_aug = [singles.tile([KA, HW], F32R, name=f"mk_aug{b}") for b in range(BL)]
        qk_aug = [singles.tile([KA, HW], F32R, name=f"qk_aug{b}") for b in range(BL)]
        ones_b = singles.tile([128, 128], BF16, name="ones_b")
        # selector stationary: ones in column CK=64 only, so the bias-row
        # matmul writes its sums straight onto partition 64 of PSUM
        sel = singles.tile([CK, 128], F32R, name="sel")
        sel_f32 = singles.tile([CK, 128], F32, name="sel_f32")
        ones_row = singles.tile([1, HW], F32, name="ones_row")
        dummy = singles.tile([1, 1], F32, name="dummy")

        prep = ctx.enter_context(tc.tile_pool(name="prep", bufs=1))
        e_pool = ctx.enter_context(tc.tile_pool(name="e_sb", bufs=3))
        o_pool = ctx.enter_context(tc.tile_pool(name="o_sb", bufs=6))
        r_pool = ctx.enter_context(tc.tile_pool(name="r", bufs=3))
        psum_e = ctx.enter_context(tc.tile_pool(name="psum_e", bufs=2, space="PSUM"))
        psum_s = ctx.enter_context(tc.tile_pool(name="psum_s", bufs=1, space="PSUM"))
        psum_a = ctx.enter_context(tc.tile_pool(name="psum_a", bufs=1, space="PSUM"))

        mk_flat = mk_ext.rearrange("b c n -> (b c) n")
        qk_flat = qk_ext.rearrange("b c n -> (b c) n")

        # ---- prep: input DMAs + bias row (-|mk_m|^2/2) via matmul ----
        # per-512-column pieces so the first chunk's matmuls start ~2.5 us in
        nc.vector.memset(sel_f32[:], 0.0)
        nc.vector.memset(sel_f32[:, CK : CK + 1], 1.0)
        nc.vector.tensor_copy(sel[:], sel_f32[:])
        nc.gpsimd.memset(ones_row[0:1, 0:256], 1.0)
        nc.gpsimd.memset(ones_b[:], 1.0)
        sq2 = [prep.tile([CK, HW], F32R, name=f"sq2_{b}") for b in range(BL)]
        a_ps = psum_a.tile([128, QCH], F32, name="a_ps")

        def emit_arow(b, c0, cw):
            # the single lhsT column is placed at partition CK=64, so the
            # bias row lands on the partition mk_aug row 64 lives on
            nc.tensor.matmul(
                a_ps[:, 0:cw],
                lhsT=sel[:],
                rhs=sq2[b][:, c0 : c0 + cw],
                start=True,
                stop=True,
            )
            nc.vector.tensor_copy(
                mk_aug[b][CK : CK + 1, c0 : c0 + cw], a_ps[CK : CK + 1, 0:cw]
            )

        def emit_prep(b, split):
            mkf = mk_flat[b * CK : (b + 1) * CK, :]
            qkf = qk_flat[b * CK : (b + 1) * CK, :]
            if split:
                nc.gpsimd.tensor_copy(
                    qk_aug[b][CK : CK + 1, 0:256], ones_row[0:1, 0:256]
                )
                nc.gpsimd.dma_start(qk_aug[b][0:CK, 0:256], qkf[:, 0:256])
                nc.gpsimd.memset(ones_row[0:1, 256:], 1.0)
                nc.gpsimd.tensor_copy(
                    qk_aug[b][CK : CK + 1, 256:], ones_row[0:1, 256:]
                )
            else:
                nc.sync.dma_start(qk_aug[b][0:CK, :], qkf[:])
                nc.gpsimd.tensor_copy(qk_aug[b][CK : CK + 1, :], ones_row[:])
            first = True
            for c0, cw in CHUNKS if split else [(0, HW)]:
                nc.sync.dma_start(
                    mk_aug[b][0:CK, c0 : c0 + cw], mkf[:, c0 : c0 + cw]
                )
                if first and b == 0:
                    # tiny exp so the ACT table set loads during the DMAs
                    nc.scalar.activation(
                        dummy[:], ones_row[0:1, 0:1], AF.Exp
                    )
                first = False
                nc.vector.scalar_tensor_tensor(
                    out=sq2[b][:, c0 : c0 + cw],
                    in0=mk_aug[b][0:CK, c0 : c0 + cw].bitcast(F32),
                    scalar=-0.5,
                    in1=mk_aug[b][0:CK, c0 : c0 + cw].bitcast(F32),
                    op0=ALU.mult,
                    op1=ALU.mult,
                )
            if split:
                emit_arow(b, *CHUNKS[0])
                nc.sync.dma_start(qk_aug[b][0:CK, 256:], qkf[:, 256:])
            else:
                for c0, cw in CHUNKS:
                    emit_arow(b, c0, cw)

        # ---- main: one softmax pipeline per (batch, q-chunk) ----
        _dma_rr = [0]

        def emit_chunk(b, q0, w, pair_hook=None, last=False):
            e_sb = e_pool.tile([128, MT, QCH], BF16, name="e_sb")
            s_ps = psum_s.tile([128, QCH], F32, name="s_ps")

            def sum_mm(t):
                # s[q] broadcast to all partitions via all-ones bf16 lhsT,
                # accumulated over the 18 m-tiles
                nc.tensor.matmul(
                    s_ps[:, 0:w],
                    lhsT=ones_b[:],
                    rhs=e_sb[:, t, 0:w],
                    start=(t == 0),
                    stop=(t == MT - 1),
                )

            G = 3
            for tp in range(MT // G):
                e_ps = psum_e.tile([128, G * QCH], F32, name="e_ps")
                ev = e_ps[:].rearrange("p (u q) -> p u q", u=G)
                for u in range(G):
                    t = G * tp + u
                    nc.tensor.matmul(
                        e_ps[:, u * QCH : u * QCH + w],
                        lhsT=mk_aug[b][:, t * 128 : (t + 1) * 128],
                        rhs=qk_aug[b][:, q0 : q0 + w],
                        start=True,
                        stop=True,
                    )
                nc.scalar.activation(
                    e_sb[:, G * tp : G * tp + G, 0:w], ev[:, :, 0:w], AF.Exp,
                    scale=0.25,
                )
                if pair_hook is not None:
                    pair_hook(tp)
                if tp >= 1:
                    for u in range(G):
                        sum_mm(G * (tp - 1) + u)
            for u in range(G):
                sum_mm(MT - G + u)

            r_sb = r_pool.tile([128, QCH], F32, name="r_sb")
            nc.vector.reciprocal_approx_fast(r_sb[:, 0:w], s_ps[:, 0:w])
            rb2 = r_pool.tile([128, 2, QCH], BF16, name="rb2")
            for u in range(2):
                nc.vector.tensor_copy(rb2[:, u, 0:w], r_sb[:, 0:w])
            for g in range(MT // 6):
                o_sb = o_pool.tile([128, 6, QCH], F16, name="o_sb")
                for pp in range(3):
                    t = 6 * g + 2 * pp
                    nc.vector.tensor_mul(
                        o_sb[:, 2 * pp : 2 * pp + 2, 0:w],
                        e_sb[:, t : t + 2, 0:w],
                        rb2[:, :, 0:w],
                    )
                dest = out_ext[
                    b, 6 * g * 128 : (6 * g + 6) * 128, q0 : q0 + w
                ].rearrange("(u p) q -> p u q", u=6)
                if last:
                    eng = (nc.sync, nc.gpsimd, nc.scalar)[g % 3]
                else:
                    eng = (nc.sync, nc.gpsimd)[_dma_rr[0] % 2]
                    _dma_rr[0] += 1
                eng.dma_start(dest, o_sb[:, :, 0:w])

        emit_prep(0, split=True)

        def chunk0_hook(tp):
            # bias-row piece k feeds tiles 4k..; emit piece tp+1 right after
            # triple tp (triple tp+1 reads rows < (3tp+6)*128 <= (tp+2)*512)
            if tp < 4:
                emit_arow(0, *CHUNKS[tp + 1])

        for i, (q0, w) in enumerate(CHUNKS_B0):
            emit_chunk(0, q0, w, pair_hook=chunk0_hook if i == 0 else None)
            if i == 0:
                emit_prep(1, split=False)
        for i, (q0, w) in enumerate(CHUNKS_B1):
            emit_chunk(1, q0, w, last=(i >= len(CHUNKS_B1) - 2))


_CACHE = {}


def _get_compiled(niter: int = 1):
    """Build+compile the per-core graph. niter>1 wraps the body in a For_i
    hardware loop (identical I/O each iteration) for differential timing."""
    key = ("nc", niter)
    if key not in _CACHE:
        nc = bacc.Bacc("TRN2", target_bir_lowering=False, debug=False)
        mk_ext = nc.dram_tensor("mk", [BL, CK, HW], F32R, kind="ExternalInput").ap()
        qk_ext = nc.dram_tensor("qk", [BL, CK, HW], F32R, kind="ExternalInput").ap()
        out_ext = nc.dram_tensor("out", [BL, HW, HW], F16, kind="ExternalOutput").ap()
        with tile.TileContext(nc) as tc:
            if niter == 1:
                _build_kernel(tc, out_ext, mk_ext, qk_ext)
            else:
                with tc.For_i(0, niter, 1):
                    _build_kernel(tc, out_ext, mk_ext, qk_ext)
        nc.compile()
        _CACHE[key] = nc
    return _CACHE[key]


class _CachedRunner:
    """Compile/upload the executable once; reuse the jitted callable for
    every subsequent call (a fresh jax.jit per call re-lowers and re-loads
    the NEFF-embedding executable through the axon tunnel each time)."""

    def __init__(self, nc, n_cores: int):
        import jax
        from jax.sharding import Mesh, PartitionSpec, NamedSharding
        from jax.experimental.shard_map import shard_map
        from concourse import bass2jax

        bass2jax.install_neuronx_cc_hook()
        self.n_cores = n_cores
        partition_name = (
            nc.partition_id_tensor.name if nc.partition_id_tensor else None
        )
        in_names, out_names, out_avals, zero_outs = [], [], [], []
        for alloc in nc.m.functions[0].allocations:
            if not isinstance(alloc, mybir.MemoryLocationSet):
                continue
            name = alloc.memorylocations[0].name
            if alloc.kind == "ExternalInput":
                if name != partition_name:
                    in_names.append(name)
            elif alloc.kind == "ExternalOutput":
                out_names.append(name)
                shape = tuple(alloc.tensor_shape)
                dtype = mybir.dt.np(alloc.dtype)
                out_avals.append(jax.core.ShapedArray(shape, dtype))
                zero_outs.append(np.zeros(shape, dtype))
        n_params = len(in_names)
        in_names = in_names + out_names
        if partition_name is not None:
            in_names.append(partition_name)
        self.in_names, self.out_names = in_names, out_names
        self.n_params, self.out_avals = n_params, out_avals

        def _body(*args):
            operands = list(args)
            if partition_name is not None:
                operands.append(bass2jax.partition_id_tensor())
            return tuple(
                bass2jax._bass_exec_p.bind(
                    *operands,
                    out_avals=tuple(out_avals),
                    in_names=tuple(in_names),
                    out_names=tuple(out_names),
                    lowering_input_output_aliases=(),
                    sim_require_finite=True,
                    sim_require_nnan=True,
                    nc=nc,
                )
            )

        P = PartitionSpec
        mesh = Mesh(np.asarray(jax.devices()[:n_cores]), ("core",))
        self.fn = jax.jit(
            shard_map(
                _body,
                mesh=mesh,
                in_specs=(P("core"),) * (n_params + len(out_names)),
                out_specs=(P("core"),) * len(out_names),
                check_rep=False,
            ),
            keep_unused=True,
        )
        sharding = NamedSharding(mesh, P("core"))
        self.zeros_dev = [
            jax.device_put(
                np.zeros((n_cores * z.shape[0], *z.shape[1:]), z.dtype), sharding
            )
            for z in zero_outs
        ]

    def __call__(self, in_maps):
        concat_in = [
            np.concatenate([np.asarray(m[name]) for m in in_maps], axis=0)
            for name in self.in_names[: self.n_params]
        ]
        out_arrs = self.fn(*concat_in, *self.zeros_dev)
        # materialize each output ONCE (np.asarray on a sharded array
        # re-gathers the full array every call)
        host = [
            np.asarray(a).reshape(self.n_cores, *self.out_avals[i].shape)
            for i, a in enumerate(out_arrs)
        ]
        return [
            {name: host[i][c] for i, name in enumerate(self.out_names)}
            for c in range(self.n_cores)
        ]


def _get_runner(niter: int = 1) -> "_CachedRunner":
    key = ("runner", niter)
    if key not in _CACHE:
        _CACHE[key] = _CachedRunner(_get_compiled(niter), NCORES)
    return _CACHE[key]


def run_spmd(mk: np.ndarray, qk: np.ndarray, niter: int = 1) -> np.ndarray:
    mk = np.ascontiguousarray(np.asarray(mk, dtype=np.float32).reshape(B, CK, HW))
    qk = np.ascontiguousarray(np.asarray(qk, dtype=np.float32).reshape(B, CK, HW))
    in_maps = [
        {"mk": mk[c * BL : (c + 1) * BL], "qk": qk[c * BL : (c + 1) * BL]}
        for c in range(NCORES)
    ]
    res = _get_runner(niter)(in_maps)
    out = np.concatenate([res[c]["out"] for c in range(NCORES)], axis=0)
    return out.reshape(B, HW, HW)


def kernel(mk: np.ndarray, qk: np.ndarray) -> np.ndarray:
    return run_spmd(mk, qk, niter=1).astype(np.float32)
